# revision 27
# baseline (speedup 1.0000x reference)
"""Trainium2 Bass kernel for nn_C2f_DualModal_MoE (top-1 MoE over 1x1 convs).

Reference computation (per token t of N = B*H*W, channels C1 -> C2):
    logits = t @ Wr                  [N, E=4]
    idx    = argmax(softmax(logits)) = argmax(logits)   (top_k = 1)
    out    = t @ We[idx] + t @ Ws[0] = t @ (We[idx] + Ws[0])

Strategy (8 NeuronCores, data-parallel over batch, 1 image per core):
  - Everything stays channel-major: x[b] is [C1, H*W] in DRAM which is
    exactly the lhs-transposed / stream layout the PE wants. The output
    [C2, H*W] is produced directly in its DRAM layout.
  - Shared expert folded into the routed weights: W'_e = We[e] + Ws[0].
  - Top-1 selection is folded into the GEMMs with a bilinear bit-mask
    decomposition. With idx = 2a + b (a,b in {0,1}):
        W'_idx = M0 + a*Ma + b*Mb + (ab)*Mab
        M0 = We0 + Ws,  Ma = We2 - We0,  Mb = We1 - We0,
        Mab = We3 - We2 - We1 + We0   (Ws cancels in the differences)
    so   out = x@M0 + (a.x)@Ma + (b.x)@Mb + (ab.x)@Mab
    with a/b per-token {0,1} masks broadcast along channels. All four
    matmuls accumulate into one PSUM tile -> no output combine.
  - Router runs in exact fp32 (token-major logits via x-stationary
    matmuls); expert GEMMs run in bf16 with fp32 PSUM accumulation.
  - Mask channel-broadcast is fully on-chip: token-major bit tile
    [128, 2*RST] -> PE transpose -> [2*RST, 128] PSUM -> ACT copy ->
    one tiny SBUF->SBUF HWDGE gather -> rows [2, BLK] -> PE ones-matmul
    broadcast -> PSUM masks that the DVE stream-masking reads directly.
  - Emission is software-pipelined: router/mask chain for block b is
    interleaved with the expert GEMMs of block b-1 so the PE never
    stalls on the mask-chain latency and LDWEIGHTS hide under streams.
"""

import contextlib

import numpy as np

import concourse.bacc as bacc
import concourse.bass as bass
import concourse.mybir as mybir
import concourse.tile as tile
from concourse import bass_utils

P = 128
B = 8
C1 = 256
C2 = 256
E = 4
HW = 6400  # 80 * 80
KO = C1 // P  # k blocks (contraction)
MO = C2 // P  # m blocks (output channels)

NBLK = 10            # token blocks per image
BLK = HW // NBLK     # 640 tokens per block
RST = BLK // P       # 5 router sub-chunks of 128 contiguous tokens
CHUNK = 320          # GEMM token chunk (2 per block, 1 PSUM bank each)
NCH = BLK // CHUNK

F32 = mybir.dt.float32
BF16 = mybir.dt.bfloat16

ALU = mybir.AluOpType
ACTF = mybir.ActivationFunctionType

# Mask broadcast into bf16 PSUM via a transpose-mode matmul (lets the DVE
# read masks straight from PSUM at 2x). If transpose-mode turns out to be
# pure data movement on HW (ignoring rhs values), flip to False: regular
# f32-PSUM matmuls + explicit bf16 copies to SBUF.
BCAST_TMODE = False


def _emit_once(nc, tc, pools, aps, rep):
    (pp, rpool, spool, opool, psum, rpsum, tpsum, bcps, bcsb) = pools
    x, wr, we, ws, out = aps

    # ---------------- x (resident, fp32, block-major) ----------------
    x_sb = pp.tile([P, NBLK, KO, BLK], F32, tag="x_sb", name=f"x_sb_{rep}")
    xv = x.rearrange("(ko ki) f -> ki ko f", ki=P)

    wr_sb = pp.tile([P, KO, E], F32, tag="wr_sb", name=f"wr_sb_{rep}")
    nc.scalar.dma_start(wr_sb[:], wr.rearrange("(ko ki) e -> ki ko e", ki=P))
    # x block 0 first (unblocks the router), then the weights (unblock the
    # expert-weight prep), then the remaining x blocks. All on the sync
    # queue so the scalar queue stays empty for the latency-critical
    # per-block mask gathers.
    nc.sync.dma_start(x_sb[:, 0], xv[:, :, 0:BLK])
    we_sb = pp.tile([P, E, KO, C2], F32, tag="we_sb", name=f"we_sb_{rep}")
    ws_sb = pp.tile([P, KO, C2], F32, tag="ws_sb", name=f"ws_sb_{rep}")
    wev = we.rearrange("e (ko ki) d -> ki e ko d", ki=P)
    nc.sync.dma_start(ws_sb[:], ws.rearrange("s (ko ki) d -> ki (s ko) d", ki=P))
    nc.sync.dma_start(we_sb[:, 0], wev[:, 0])
    nc.sync.dma_start(we_sb[:, 2], wev[:, 2])
    nc.sync.dma_start(x_sb[:, 1], xv[:, :, BLK : 2 * BLK])
    nc.sync.dma_start(we_sb[:, 1], wev[:, 1])
    nc.sync.dma_start(we_sb[:, 3], wev[:, 3])
    for b in range(2, NBLK):
        nc.sync.dma_start(x_sb[:, b], xv[:, :, b * BLK : (b + 1) * BLK])

    # bilinear matrices in bf16 (stream s: 0->M0, 1->Ma, 2->Mb, 3->Mab).
    # msb0-2 prep on the otherwise-idle GPSIMD engine in consumption
    # order; msb3's 3-op chain runs on DVE (emitted inside iteration 0,
    # after block 0's router bits, so it doesn't block the mask chain).
    msb = pp.tile([P, 4, KO, C2], BF16, tag="msb", name=f"msb_{rep}")
    t1_f = pp.tile([P, KO, C2], F32, tag="t1_f", name=f"t1_f_{rep}")
    nc.gpsimd.tensor_tensor(msb[:, 0], we_sb[:, 0], ws_sb[:], ALU.add)
    nc.gpsimd.tensor_tensor(msb[:, 1], we_sb[:, 2], we_sb[:, 0], ALU.subtract)
    nc.gpsimd.tensor_tensor(msb[:, 2], we_sb[:, 1], we_sb[:, 0], ALU.subtract)

    def emit_msb3():
        nc.vector.tensor_tensor(t1_f[:], we_sb[:, 3], we_sb[:, 2], ALU.subtract)
        nc.vector.tensor_tensor(t1_f[:], t1_f[:], we_sb[:, 1], ALU.subtract)
        nc.vector.tensor_tensor(msb[:, 3], t1_f[:], we_sb[:, 0], ALU.add)

    # descending weights [4,3,2,1] pick the FIRST argmax on ties
    w4 = pp.tile([P, E], F32, tag="w4", name=f"w4_{rep}")
    for j in range(E):
        nc.vector.memset(w4[:, j : j + 1], float(E - j))

    # identity (bf16) for PE transposes; row-selectors for the broadcast
    eye = pp.tile([P, P], BF16, tag="eye", name=f"eye_{rep}")
    ones = pp.tile([P, P], BF16, tag="ones", name=f"ones_{rep}")
    nc.vector.memset(ones[:], 1.0)
    nc.gpsimd.affine_select(
        eye[:], ones[:], [[1, P]], ALU.is_equal, 0.0, base=0, channel_multiplier=-1
    )
    # sel[:, 0] = [[1...],[0...]] selects row a; sel[:, 1] = ones - sel0
    # selects row b (a lone memset of partition 1 is not a legal DVE AP)
    sel = pp.tile([2, 2, P], BF16, tag="sel", name=f"sel_{rep}")
    nc.vector.memset(sel[:, 0], 0.0)
    nc.vector.memset(sel[0:1, 0], 1.0)
    nc.vector.tensor_tensor(sel[:, 1], ones[0:2, :], sel[:, 0], ALU.subtract)

    out_v = out.rearrange("(mo mi) f -> mi mo f", mi=P)

    # ---------------- software-pipelined main loop ----------------
    # Iteration b emits: router+mask chain for block b, expert GEMMs for
    # block b-1 (with block b's broadcast matmuls slotted between expert
    # chunks so the mask-gather DMA latency hides under expert streams).
    rws = [None] * NBLK   # row tiles [2, BLK]
    bcts = [None] * NBLK  # broadcast mask PSUM tiles

    def emit_router(b):
        # logits, token-major: [128 tokens, RST, E]; sub-chunk i covers
        # the 128 contiguous tokens [i*128, (i+1)*128)
        pr = rpsum.tile([P, RST, E], F32, tag="pr", name=f"pr_{rep}_{b}")
        for i in range(RST):
            for k in range(KO):
                nc.tensor.matmul(
                    pr[:, i, :],
                    x_sb[:, b, k, i * P : (i + 1) * P],
                    wr_sb[:, k, :],
                    start=(k == 0),
                    stop=(k == KO - 1),
                )
        lg = rpool.tile([P, RST, E], F32, tag="lg", name=f"lg_{rep}_{b}")
        nc.vector.tensor_copy(lg[:], pr[:])
        mx = rpool.tile([P, RST], F32, tag="mx", name=f"mx_{rep}_{b}")
        nc.vector.reduce_max(mx[:], lg[:], axis=mybir.AxisListType.X)
        eq = rpool.tile([P, RST, E], F32, tag="eq", name=f"eq_{rep}_{b}")
        nc.vector.tensor_tensor(
            eq[:], lg[:], mx[:, :, None].to_broadcast((P, RST, E)), ALU.is_equal
        )
        nc.vector.tensor_tensor(
            eq[:], eq[:], w4[:, None, :].to_broadcast((P, RST, E)), ALU.mult
        )
        # rmax = 4 - idx; a = (idx>=2) = (rmax<=2); b_bit = idx%2 = rmax%2
        rmax = rpool.tile([P, RST], F32, tag="rmax", name=f"rmax_{rep}_{b}")
        nc.vector.reduce_max(rmax[:], eq[:], axis=mybir.AxisListType.X)
        abm = rpool.tile([P, 2, RST], BF16, tag="abm", name=f"abm_{rep}_{b}")
        nc.vector.tensor_scalar(abm[:, 0], rmax[:], 2.0, None, ALU.is_le)
        idx = rpool.tile([P, RST], F32, tag="idx", name=f"idx_{rep}_{b}")
        nc.vector.tensor_scalar(idx[:], rmax[:], -1.0, float(E), ALU.mult, ALU.add)
        a_f = rpool.tile([P, RST], F32, tag="a_f", name=f"a_f_{rep}_{b}")
        nc.vector.tensor_scalar(a_f[:], idx[:], 2.0, -2.0, ALU.is_ge, ALU.mult)
        nc.vector.tensor_tensor(abm[:, 1], a_f[:], idx[:], ALU.add)

        # PE transpose -> [(j r), 128] PSUM (bf16), ACT copy to SBUF,
        # one 10-descriptor SBUF->SBUF gather -> token rows [2, BLK]
        tps = tpsum.tile([2 * RST, P], BF16, tag="tps", name=f"tps_{rep}_{b}")
        nc.tensor.transpose(tps[:], abm[:].rearrange("p j r -> p (j r)"), eye[:])
        mt = rpool.tile([2 * RST, P], BF16, tag="mt", name=f"mt_{rep}_{b}")
        nc.scalar.activation(mt[:], tps[:], ACTF.Copy)
        # scalar HWDGE queue: not stuck in FIFO order behind the bulk x
        # loads on the sync queue (out stores here are small and frequent)
        rw = rpool.tile([2, BLK], BF16, tag="rw", name=f"rw_{rep}_{b}")
        nc.scalar.dma_start(rw[:], mt[:])
        rws[b] = rw

    def emit_bcast(b):
        # broadcast both mask rows across all 128 partitions via
        # ones-matmuls (one full PSUM bank per (mask, chunk) -- a matmul
        # target must not straddle banks and psum pool tags/bufs pack
        # contiguously), then stage to SBUF bf16 (ACT + DVE split)
        bc2 = bcsb.tile([P, 2, BLK], BF16, tag="bc2", name=f"bc2_{rep}_{b}")
        for j in range(2):
            for c in range(NCH):
                csl = slice(c * CHUNK, (c + 1) * CHUNK)
                bct = bcps.tile(
                    [P, 512], F32, tag=f"bct{j}{c}", name=f"bct{j}{c}_{rep}_{b}"
                )
                nc.tensor.matmul(
                    bct[:, :CHUNK], sel[:, j], rws[b][:, csl], start=True, stop=True
                )
                if (j + c) % 2 == 0:
                    nc.scalar.activation(bc2[:, j, csl], bct[:, :CHUNK], ACTF.Copy)
                else:
                    nc.vector.tensor_copy(bc2[:, j, csl], bct[:, :CHUNK])
        bcts[b] = bc2

    def emit_expert_chunk(b, c):
        lsl = slice(c * CHUNK, (c + 1) * CHUNK)
        bc2 = bcts[b]
        xs = spool.tile([P, KO, CHUNK], BF16, tag="xs", name=f"xs_{rep}_{b}_{c}")
        if c % 2 == 0:
            nc.vector.tensor_copy(xs[:], x_sb[:, b, :, lsl])
        else:
            nc.scalar.activation(xs[:], x_sb[:, b, :, lsl], ACTF.Copy)
        xa = spool.tile([P, KO, CHUNK], BF16, tag="xa", name=f"xa_{rep}_{b}_{c}")
        nc.vector.tensor_tensor(
            xa[:], xs[:], bc2[:, 0, None, lsl].to_broadcast((P, KO, CHUNK)), ALU.mult
        )
        xb = spool.tile([P, KO, CHUNK], BF16, tag="xb", name=f"xb_{rep}_{b}_{c}")
        nc.vector.tensor_tensor(
            xb[:], xs[:], bc2[:, 1, None, lsl].to_broadcast((P, KO, CHUNK)), ALU.mult
        )
        xab = spool.tile([P, KO, CHUNK], BF16, tag="xab", name=f"xab_{rep}_{b}_{c}")
        nc.vector.tensor_tensor(
            xab[:], xb[:], bc2[:, 0, None, lsl].to_broadcast((P, KO, CHUNK)), ALU.mult
        )
        streams = [xs, xa, xb, xab]
        for m in range(MO):
            # padded to a full bank: a matmul target must not straddle banks
            po = psum.tile([P, 512], F32, tag="po", name=f"po_{rep}_{b}_{c}_{m}")
            for s in range(4):
                for k in range(KO):
                    nc.tensor.matmul(
                        po[:, :CHUNK],
                        msb[:, s, k, m * P : (m + 1) * P],
                        streams[s][:, k, :],
                        start=(s == 0 and k == 0),
                        stop=(s == 3 and k == KO - 1),
                    )
            stg = opool.tile([P, CHUNK], F32, tag=f"stg{m}", name=f"stg{m}_{rep}_{b}_{c}")
            nc.scalar.activation(stg[:], po[:, :CHUNK], ACTF.Copy)
            nc.scalar.dma_start(out_v[:, m, b * BLK + c * CHUNK : b * BLK + (c + 1) * CHUNK], stg[:])

    # Offset-2 software pipeline: iteration b runs the router for block b,
    # the mask broadcast for block b-1 (whose gather DMA has had a full
    # period to land), and the expert GEMMs for block b-2.
    # Per-iteration PE order: expert chunk 0 of block b-2 first (its
    # inputs are always ready), then the router for block b (absorbs x
    # arrival jitter during the ramp), then the broadcast for block b-1
    # (its gather DMA is >1 period old), then expert chunk 1.
    for b in range(NBLK + 2):
        if b >= 2:
            emit_expert_chunk(b - 2, 0)
        if b < NBLK:
            emit_router(b)
        if b == 0:
            emit_msb3()
        if 1 <= b <= NBLK:
            emit_bcast(b - 1)
        if b >= 2:
            emit_expert_chunk(b - 2, 1)


def _build_body(tc, x, wr, we, ws, out, loop_n=None):
    nc = tc.nc
    ctx = contextlib.ExitStack()
    with ctx:
        pp = ctx.enter_context(tc.tile_pool(name="persist", bufs=1))
        rpool = ctx.enter_context(tc.tile_pool(name="router", bufs=3))
        spool = ctx.enter_context(tc.tile_pool(name="streams", bufs=4))
        opool = ctx.enter_context(tc.tile_pool(name="outstg", bufs=6))
        psum = ctx.enter_context(tc.tile_pool(name="psum", bufs=2, space="PSUM"))
        rpsum = ctx.enter_context(tc.tile_pool(name="rpsum", bufs=1, space="PSUM"))
        tpsum = ctx.enter_context(tc.tile_pool(name="tpsum", bufs=1, space="PSUM"))
        bcps = ctx.enter_context(tc.tile_pool(name="bcpsum", bufs=1, space="PSUM"))
        bcsb = ctx.enter_context(tc.tile_pool(name="bcsb", bufs=3))
        pools = (pp, rpool, spool, opool, psum, rpsum, tpsum, bcps, bcsb)
        if isinstance(loop_n, str) and loop_n.startswith("for"):
            n_iter = int(loop_n[3:])
            with tc.For_i(0, n_iter, 1):
                _emit_once(nc, tc, pools, (x, wr, we, ws, out), 0)
        else:
            for rep in range(loop_n or 1):
                _emit_once(nc, tc, pools, (x, wr, we, ws, out), rep)


_NC_CACHE = {}


def _get_nc(loop_n=None):
    key = ("nc", loop_n)
    if key not in _NC_CACHE:
        nc = bacc.Bacc("TRN2", debug=False, num_swdge_queues=4)
        x = nc.dram_tensor("x", [C1, HW], F32, kind="ExternalInput").ap()
        wr = nc.dram_tensor("wr", [C1, E], F32, kind="ExternalInput").ap()
        we = nc.dram_tensor("we", [E, C1, C2], F32, kind="ExternalInput").ap()
        ws = nc.dram_tensor("ws", [1, C1, C2], F32, kind="ExternalInput").ap()
        out = nc.dram_tensor("out", [C2, HW], F32, kind="ExternalOutput").ap()
        with tile.TileContext(nc) as tc:
            _build_body(tc, x, wr, we, ws, out, loop_n=loop_n)
        nc.compile()
        _NC_CACHE[key] = nc
    return _NC_CACHE[key]


def kernel(x, Wr, We, Ws, top_k, _trace=False):
    assert int(top_k) == 1, "kernel hardcodes top_k == 1"
    x = np.ascontiguousarray(np.asarray(x, dtype=np.float32))
    Wr_n = np.ascontiguousarray(np.asarray(Wr, dtype=np.float32))
    We_n = np.ascontiguousarray(np.asarray(We, dtype=np.float32))
    Ws_n = np.ascontiguousarray(np.asarray(Ws, dtype=np.float32))
    b, c1, h, w = x.shape
    assert (b, c1, h * w) == (B, C1, HW)

    nc = _get_nc()
    in_maps = [
        {
            "x": x[core].reshape(C1, HW),
            "wr": Wr_n,
            "we": We_n,
            "ws": Ws_n,
        }
        for core in range(B)
    ]
    res = bass_utils.run_bass_kernel_spmd(
        nc, in_maps, core_ids=list(range(B)), trace=_trace
    )
    outs = np.stack([res.results[core]["out"] for core in range(B)])
    out = outs.reshape(B, C2, h, w)
    if _trace:
        return out, res
    return out
